# revision 3
# baseline (speedup 1.0000x reference)
"""Trainium2 Bass kernel: 2-layer GRU decoder step with a 32k-vocab head.

Sharding (8 NeuronCores, one chip):
  * GRU layers are tensor-parallel over the hidden dim: core i owns hidden
    units [i*128, (i+1)*128) of each layer, holding the matching column
    slices of Wi/Wr (gates z/r/h re-packed side by side). After each layer
    the per-core transposed state slices are AllGathered so the next stage
    has the full hidden state as a stationary operand.
  * The vocab head is tensor-parallel over the vocab dim: core i owns
    ffW[:, i*4000:(i+1)*4000] and produces logits[:, i*4000:(i+1)*4000].
  * The embedding gather runs on-device via indirect DMA (data-dependent).
Host side does layout-only staging: slicing, transposes, dtype casts.
"""

import numpy as np

B = 128
EMB = 256
H = 1024
V = 32000
NCORES = 8
HS = H // NCORES          # 128 hidden units per core per layer
VS = V // NCORES          # 4000 vocab columns per core
NT = 8                    # head column tiles per core (4000 = 8*500)
NTS = VS // NT            # 500

# Weight/matmul-operand dtype: "f32" or "bf16".  F32R additionally bitcasts
# fp32 matmul operands to float32r (single-pass PE matmul).
W_DT = "f32"
F32R = False
FW_BUFS = 6

_built = None


def _gate_cols(W, i):
    """Column slice of a [in, 3*H] GRU weight for hidden shard i, gates
    re-packed as [z(128) | r(128) | h(128)]."""
    return np.concatenate(
        [W[:, g * H + i * HS : g * H + (i + 1) * HS] for g in range(3)], axis=1
    )


def _build_kernel(wdt_np):
    from concourse import mybir, tile, bacc
    import concourse.bass as bass
    from concourse.masks import make_identity

    f32 = mybir.dt.float32
    wdt = mybir.dt.from_np(wdt_np)
    is_bf16 = wdt != f32

    def mmdt(ap):
        if F32R and not is_bf16:
            return ap.bitcast(mybir.dt.float32r)
        return ap

    nc = bacc.Bacc(
        "TRN2", target_bir_lowering=False, debug=False, num_devices=NCORES
    )

    # ---- DRAM I/O ----
    x_ids = nc.dram_tensor("x_ids", [B, 1], mybir.dt.int32, kind="ExternalInput")
    emb = nc.dram_tensor("emb", [V, EMB], wdt, kind="ExternalInput")
    h0T_d = nc.dram_tensor("h0T", [H, B], wdt, kind="ExternalInput")
    h1T_d = nc.dram_tensor("h1T", [H, B], wdt, kind="ExternalInput")
    h0sl_d = nc.dram_tensor("h0sl", [B, HS], f32, kind="ExternalInput")
    h1sl_d = nc.dram_tensor("h1sl", [B, HS], f32, kind="ExternalInput")
    wi0_d = nc.dram_tensor("wi0", [EMB, 3 * HS], wdt, kind="ExternalInput")
    wr0_d = nc.dram_tensor("wr0", [H, 3 * HS], wdt, kind="ExternalInput")
    wi1_d = nc.dram_tensor("wi1", [H, 3 * HS], wdt, kind="ExternalInput")
    wr1_d = nc.dram_tensor("wr1", [H, 3 * HS], wdt, kind="ExternalInput")
    b0_d = nc.dram_tensor("b0", [1, 4 * HS], wdt, kind="ExternalInput")
    b1_d = nc.dram_tensor("b1", [1, 4 * HS], wdt, kind="ExternalInput")
    ffw_d = nc.dram_tensor("ffw", [H, VS], wdt, kind="ExternalInput")
    ffb_d = nc.dram_tensor("ffb", [1, VS], wdt, kind="ExternalInput")

    logits_o = nc.dram_tensor("logits_s", [B, VS], f32, kind="ExternalOutput")
    h0n_o = nc.dram_tensor("h0n_s", [B, HS], f32, kind="ExternalOutput")
    h1n_o = nc.dram_tensor("h1n_s", [B, HS], f32, kind="ExternalOutput")

    P = 128
    RG = [list(range(NCORES))]

    with tile.TileContext(nc) as tc:
        with (
            tc.tile_pool(name="const", bufs=1) as cpool,
            tc.tile_pool(name="wts", bufs=1) as wpool,
            tc.tile_pool(name="acts", bufs=2) as apool,
            tc.tile_pool(name="ffp", bufs=FW_BUFS) as ffp,
            tc.tile_pool(name="outp", bufs=3) as opool,
            tc.tile_pool(name="dram", bufs=2, space="DRAM") as dpool,
        ):
            # ---- constants ----
            ident32 = cpool.tile([P, P], f32)
            make_identity(nc, ident32[:])
            if is_bf16:
                identw = cpool.tile([P, P], wdt)
                make_identity(nc, identw[:])
            else:
                identw = ident32
            ones = cpool.tile([1, P], wdt)
            nc.gpsimd.memset(ones[:], 1.0)

            # ---- load indices and gather embedding rows ----
            idx = cpool.tile([B, 1], mybir.dt.int32)
            nc.sync.dma_start(idx[:], x_ids[:])
            e_sb = cpool.tile([P, EMB], wdt)
            nc.gpsimd.indirect_dma_start(
                out=e_sb[:],
                out_offset=None,
                in_=emb[:],
                in_offset=bass.IndirectOffsetOnAxis(ap=idx[:, :1], axis=0),
            )

            # ---- GRU weights / state loads (all resident) ----
            wi0_sb = wpool.tile([P, 2 * 3 * HS], wdt)
            for k in range(2):
                nc.sync.dma_start(
                    wi0_sb[:, k * 3 * HS : (k + 1) * 3 * HS],
                    wi0_d[k * P : (k + 1) * P, :],
                )
            wr0_sb = wpool.tile([P, 8 * 3 * HS], wdt)
            wi1_sb = wpool.tile([P, 8 * 3 * HS], wdt)
            wr1_sb = wpool.tile([P, 8 * 3 * HS], wdt)
            for sb, d in ((wr0_sb, wr0_d), (wi1_sb, wi1_d), (wr1_sb, wr1_d)):
                for k in range(8):
                    nc.sync.dma_start(
                        sb[:, k * 3 * HS : (k + 1) * 3 * HS],
                        d[k * P : (k + 1) * P, :],
                    )
            h0T_sb = wpool.tile([P, H], wdt)
            h1T_sb = wpool.tile([P, H], wdt)
            for sb, d in ((h0T_sb, h0T_d), (h1T_sb, h1T_d)):
                for k in range(8):
                    nc.sync.dma_start(
                        sb[:, k * P : (k + 1) * P], d[k * P : (k + 1) * P, :]
                    )
            h0sl = cpool.tile([B, HS], f32)
            nc.sync.dma_start(h0sl[:], h0sl_d[:])
            h1sl = cpool.tile([B, HS], f32)
            nc.sync.dma_start(h1sl[:], h1sl_d[:])
            b0_sb = cpool.tile([1, 4 * HS], wdt)
            nc.sync.dma_start(b0_sb[:], b0_d[:])
            b1_sb = cpool.tile([1, 4 * HS], wdt)
            nc.sync.dma_start(b1_sb[:], b1_d[:])
            ffb_sb = cpool.tile([1, VS], wdt)
            nc.sync.dma_start(ffb_sb[:], ffb_d[:])

            with tc.tile_pool(name="psg", bufs=2, space="PSUM") as psg:
                # ---- transpose gathered embeddings: eT[k] = e[:, k*128:].T ----
                eT = cpool.tile([P, EMB], wdt)
                for k in range(2):
                    trw = psg.tile([P, P], wdt, tag="trw", bufs=1, name=f"trw{k}")
                    nc.tensor.transpose(
                        trw[:], e_sb[:, k * P : (k + 1) * P], identw[:]
                    )
                    nc.vector.tensor_copy(eT[:, k * P : (k + 1) * P], trw[:])

                def gru_layer(lname, xT_sb, nkx, wi_sb, wr_sb, hT_sb, b_sb, hsl, hn_out):
                    """One GRU layer shard; returns the transposed new-state
                    slice (wdt, SBUF [P, HS])."""
                    zr = psg.tile([B, 2 * HS], f32, tag="zr", name=f"zr_{lname}")
                    ih = psg.tile([B, HS], f32, tag="ih", name=f"ih_{lname}")
                    rh = psg.tile([B, HS], f32, tag="rh", name=f"rh_{lname}")
                    # bias init (start=True clears the accumulation groups)
                    nc.tensor.matmul(
                        zr[:], mmdt(ones[:1, :]), mmdt(b_sb[:1, 0 : 2 * HS]),
                        start=True, stop=False,
                    )
                    nc.tensor.matmul(
                        ih[:], mmdt(ones[:1, :]), mmdt(b_sb[:1, 2 * HS : 3 * HS]),
                        start=True, stop=False,
                    )
                    nc.tensor.matmul(
                        rh[:], mmdt(ones[:1, :]), mmdt(b_sb[:1, 3 * HS : 4 * HS]),
                        start=True, stop=False,
                    )
                    # x-side: gi = x @ Wi_s   (zr cols + ih col)
                    for k in range(nkx):
                        xT = mmdt(xT_sb[:, k * P : (k + 1) * P])
                        nc.tensor.matmul(
                            zr[:], xT, mmdt(wi_sb[:, k * 3 * HS : k * 3 * HS + 2 * HS]),
                            start=False, stop=False,
                        )
                        nc.tensor.matmul(
                            ih[:], xT,
                            mmdt(wi_sb[:, k * 3 * HS + 2 * HS : (k + 1) * 3 * HS]),
                            start=False, stop=(k == nkx - 1),
                        )
                    # h-side: gr = h @ Wr_s   (zr cols + rh col)
                    for k in range(8):
                        hT = mmdt(hT_sb[:, k * P : (k + 1) * P])
                        nc.tensor.matmul(
                            zr[:], hT, mmdt(wr_sb[:, k * 3 * HS : k * 3 * HS + 2 * HS]),
                            start=False, stop=(k == 7),
                        )
                        nc.tensor.matmul(
                            rh[:], hT,
                            mmdt(wr_sb[:, k * 3 * HS + 2 * HS : (k + 1) * 3 * HS]),
                            start=False, stop=(k == 7),
                        )
                    # gates
                    z = apool.tile([B, HS], f32, tag="z", name=f"z_{lname}")
                    nc.scalar.activation(
                        z[:], zr[:, 0:HS], mybir.ActivationFunctionType.Sigmoid
                    )
                    r = apool.tile([B, HS], f32, tag="r", name=f"r_{lname}")
                    nc.scalar.activation(
                        r[:], zr[:, HS : 2 * HS], mybir.ActivationFunctionType.Sigmoid
                    )
                    t = apool.tile([B, HS], f32, tag="t", name=f"t_{lname}")
                    nc.vector.tensor_mul(t[:], r[:], rh[:])
                    u = apool.tile([B, HS], f32, tag="u", name=f"u_{lname}")
                    nc.vector.tensor_add(u[:], ih[:], t[:])
                    hh = apool.tile([B, HS], f32, tag="hh", name=f"hh_{lname}")
                    nc.scalar.activation(
                        hh[:], u[:], mybir.ActivationFunctionType.Tanh
                    )
                    d = apool.tile([B, HS], f32, tag="d", name=f"d_{lname}")
                    nc.vector.tensor_sub(d[:], hsl[:], hh[:])
                    zd = apool.tile([B, HS], f32, tag="zd", name=f"zd_{lname}")
                    nc.vector.tensor_mul(zd[:], z[:], d[:])
                    hn = apool.tile([B, HS], f32, tag="hn", name=f"hn_{lname}")
                    nc.vector.tensor_add(hn[:], hh[:], zd[:])
                    nc.sync.dma_start(hn_out[:], hn[:])
                    # transpose the slice for the AllGather
                    tr = psg.tile([P, HS], f32, tag="tr", bufs=1, name=f"tr_{lname}")
                    nc.tensor.transpose(tr[:], hn[:], ident32[:])
                    hnT = apool.tile([P, HS], wdt, tag="hnT", name=f"hnT_{lname}")
                    nc.vector.tensor_copy(hnT[:], tr[:])
                    return hnT

                def allgather(lname, hnT, dst_sb):
                    agi = dpool.tile([P, HS], wdt, tag="agi", name=f"agi_{lname}")
                    nc.sync.dma_start(agi[:], hnT[:])
                    ago = dpool.tile(
                        [H, B], wdt, tag="ago", name=f"ago_{lname}",
                        addr_space="Shared",
                    )
                    nc.gpsimd.collective_compute(
                        "AllGather",
                        mybir.AluOpType.bypass,
                        replica_groups=RG,
                        ins=[agi[:].opt()],
                        outs=[ago[:].opt()],
                    )
                    for k in range(8):
                        nc.sync.dma_start(
                            dst_sb[:, k * P : (k + 1) * P],
                            ago[k * P : (k + 1) * P, :],
                        )

                h0nT_sb = wpool.tile([P, H], wdt)
                h1nT_sb = wpool.tile([P, H], wdt)

                hnT0 = gru_layer(
                    "l0", eT, 2, wi0_sb, wr0_sb, h0T_sb, b0_sb, h0sl, h0n_o
                )
                allgather("l0", hnT0, h0nT_sb)
                hnT1 = gru_layer(
                    "l1", h0nT_sb, 8, wi1_sb, wr1_sb, h1T_sb, b1_sb, h1sl, h1n_o
                )
                allgather("l1", hnT1, h1nT_sb)

            # ---- vocab head: logits_s = h1n @ ffW_s + ffb_s ----
            with tc.tile_pool(name="psh", bufs=1, space="PSUM") as psh:
                pls = [
                    psh.tile([B, NTS], f32, tag=f"pl{j}", name=f"pl{j}")
                    for j in range(NT)
                ]
                for j in range(NT):
                    nc.tensor.matmul(
                        pls[j][:], mmdt(ones[:1, :]),
                        mmdt(ffb_sb[:1, j * NTS : (j + 1) * NTS]),
                        start=True, stop=False,
                    )
                for k in range(8):
                    fw = ffp.tile([P, VS], wdt, tag="fw", name=f"fw{k}")
                    nc.sync.dma_start(fw[:], ffw_d[k * P : (k + 1) * P, :])
                    h1nT_k = mmdt(h1nT_sb[:, k * P : (k + 1) * P])
                    for j in range(NT):
                        nc.tensor.matmul(
                            pls[j][:], h1nT_k,
                            mmdt(fw[:, j * NTS : (j + 1) * NTS]),
                            start=False, stop=(k == 7),
                        )
                for j in range(NT):
                    lo = opool.tile([B, NTS], f32, tag="lo", name=f"lo{j}")
                    nc.vector.tensor_copy(lo[:], pls[j][:])
                    nc.sync.dma_start(logits_o[:, j * NTS : (j + 1) * NTS], lo[:])

    nc.compile()
    return nc


def _make_runner(nc):
    """Build a jit'd 8-core executor with pre-placed device inputs.

    Mirrors concourse.bass2jax.run_bass_via_pjrt's multi-core path, but
    exposes input placement so host->device transfers are excluded from the
    measured NEFF execution (avoids cross-core launch skew from transfer
    completion times).
    """
    import jax
    from jax.sharding import Mesh, PartitionSpec, NamedSharding
    from jax.experimental.shard_map import shard_map
    from concourse import mybir
    import concourse.bass2jax as b2j

    b2j.install_neuronx_cc_hook()

    partition_name = (
        nc.partition_id_tensor.name if nc.partition_id_tensor else None
    )
    in_names, out_names, out_avals, zero_shapes = [], [], [], []
    for alloc in nc.m.functions[0].allocations:
        if not isinstance(alloc, mybir.MemoryLocationSet):
            continue
        name = alloc.memorylocations[0].name
        if alloc.kind == "ExternalInput":
            if name != partition_name:
                in_names.append(name)
        elif alloc.kind == "ExternalOutput":
            shape = tuple(alloc.tensor_shape)
            dtype = mybir.dt.np(alloc.dtype)
            out_names.append(name)
            out_avals.append(jax.core.ShapedArray(shape, dtype))
            zero_shapes.append((shape, dtype))
    n_params = len(in_names)
    n_outs = len(out_names)
    all_in_names = list(in_names) + list(out_names)
    if partition_name is not None:
        all_in_names.append(partition_name)

    donate = tuple(range(n_params, n_params + n_outs))

    def _body(*args):
        operands = list(args)
        if partition_name is not None:
            operands.append(b2j.partition_id_tensor())
        outs = b2j._bass_exec_p.bind(
            *operands,
            out_avals=tuple(out_avals),
            in_names=tuple(all_in_names),
            out_names=tuple(out_names),
            lowering_input_output_aliases=(),
            sim_require_finite=True,
            sim_require_nnan=True,
            nc=nc,
        )
        return tuple(outs)

    devices = jax.devices()[:NCORES]
    mesh = Mesh(np.asarray(devices), ("core",))
    in_specs = (PartitionSpec("core"),) * (n_params + n_outs)
    out_specs = (PartitionSpec("core"),) * n_outs
    sharded = jax.jit(
        shard_map(
            _body, mesh=mesh, in_specs=in_specs, out_specs=out_specs,
            check_rep=False,
        ),
        donate_argnums=donate,
        keep_unused=True,
    )
    sharding = NamedSharding(mesh, PartitionSpec("core"))

    def run(in_maps):
        import jax

        concat_in = [
            np.ascontiguousarray(
                np.concatenate([np.asarray(in_maps[c][n]) for c in range(NCORES)], axis=0)
            )
            for n in in_names
        ]
        dev_in = [jax.device_put(a, sharding) for a in concat_in]
        for a in dev_in:
            a.block_until_ready()

        def call():
            zeros = [
                jax.device_put(
                    np.zeros((NCORES * s[0], *s[1:]), dt), sharding
                )
                for (s, dt) in zero_shapes
            ]
            for z in zeros:
                z.block_until_ready()
            outs = sharded(*dev_in, *zeros)
            for o in outs:
                o.block_until_ready()
            return outs

        call()  # warmup: compile + first NEFF execution
        outs = call()
        res = [
            {
                name: np.asarray(outs[i]).reshape(NCORES, *out_avals[i].shape)[c]
                for i, name in enumerate(out_names)
            }
            for c in range(NCORES)
        ]
        return res, call

    return run


def _get():
    global _built
    if _built is None:
        wdt_np = np.float32 if W_DT == "f32" else __import__("ml_dtypes").bfloat16
        nc = _build_kernel(wdt_np)
        runner = _make_runner(nc)
        _built = (nc, runner, wdt_np)
    return _built


def kernel(**inputs):
    nc, runner, wdt_np = _get()
    in_maps = _prepare_in_maps(inputs, wdt_np)
    res, _ = runner(in_maps)
    return _assemble(res)


def _prepare_in_maps(inputs, wdt_np):
    f32 = np.float32
    x = np.asarray(inputs["x"]).astype(np.int32).reshape(B, 1)
    h0 = np.asarray(inputs["h0"], dtype=f32)
    h1 = np.asarray(inputs["h1"], dtype=f32)
    emb = np.asarray(inputs["emb"], dtype=f32)
    Wi0 = np.asarray(inputs["Wi0"], dtype=f32)
    Wr0 = np.asarray(inputs["Wr0"], dtype=f32)
    bi0 = np.asarray(inputs["bi0"], dtype=f32)
    br0 = np.asarray(inputs["br0"], dtype=f32)
    Wi1 = np.asarray(inputs["Wi1"], dtype=f32)
    Wr1 = np.asarray(inputs["Wr1"], dtype=f32)
    bi1 = np.asarray(inputs["bi1"], dtype=f32)
    br1 = np.asarray(inputs["br1"], dtype=f32)
    ffW = np.asarray(inputs["ffW"], dtype=f32)
    ffb = np.asarray(inputs["ffb"], dtype=f32)

    w = lambda a: np.ascontiguousarray(a).astype(wdt_np)
    emb_w = w(emb)
    h0T = w(h0.T)
    h1T = w(h1.T)

    def layer_bias(bi, br, i):
        bsum = bi + br
        bz = bsum[0 * H + i * HS : 0 * H + (i + 1) * HS]
        br_ = bsum[1 * H + i * HS : 1 * H + (i + 1) * HS]
        bih = bi[2 * H + i * HS : 2 * H + (i + 1) * HS]
        brh = br[2 * H + i * HS : 2 * H + (i + 1) * HS]
        return w(np.concatenate([bz, br_, bih, brh])[None, :])

    in_maps = []
    for i in range(NCORES):
        in_maps.append(
            {
                "x_ids": x,
                "emb": emb_w,
                "h0T": h0T,
                "h1T": h1T,
                "h0sl": np.ascontiguousarray(h0[:, i * HS : (i + 1) * HS]),
                "h1sl": np.ascontiguousarray(h1[:, i * HS : (i + 1) * HS]),
                "wi0": w(_gate_cols(Wi0, i)),
                "wr0": w(_gate_cols(Wr0, i)),
                "wi1": w(_gate_cols(Wi1, i)),
                "wr1": w(_gate_cols(Wr1, i)),
                "b0": layer_bias(bi0, br0, i),
                "b1": layer_bias(bi1, br1, i),
                "ffw": w(ffW[:, i * VS : (i + 1) * VS]),
                "ffb": w(ffb[i * VS : (i + 1) * VS][None, :]),
            }
        )
    return in_maps


def _assemble(res):
    logits = np.concatenate([res[c]["logits_s"] for c in range(NCORES)], axis=1)
    h0n = np.concatenate([res[c]["h0n_s"] for c in range(NCORES)], axis=1)
    h1n = np.concatenate([res[c]["h1n_s"] for c in range(NCORES)], axis=1)
    return logits, h0n, h1n


# revision 6
# speedup vs baseline: 1.3637x; 1.3637x over previous
"""Trainium2 Bass kernel: 2-layer GRU decoder step with a 32k-vocab head.

Sharding (8 NeuronCores, one chip):
  * GRU layers are tensor-parallel over the hidden dim: core i owns hidden
    units [i*128, (i+1)*128) of each layer, holding the matching column
    slices of Wi/Wr (gates z/r/h re-packed side by side). After each layer
    the per-core transposed state slices are AllGathered so the next stage
    has the full hidden state as a stationary operand.
  * The vocab head is tensor-parallel over the vocab dim: core i owns
    ffW[:, i*4000:(i+1)*4000] and produces logits[:, i*4000:(i+1)*4000].
  * The embedding gather runs on-device via indirect DMA (data-dependent).
Host side does layout-only staging: slicing, transposes, dtype casts.
"""

import numpy as np

B = 128
EMB = 256
H = 1024
V = 32000
NCORES = 8
HS = H // NCORES          # 128 hidden units per core per layer
VS = V // NCORES          # 4000 vocab columns per core
NT = 8                    # head column tiles per core (4000 = 8*500)
NTS = VS // NT            # 500

# W_CFG: "bf16" (GRU + head weights bf16), "mixed" (GRU f32, head bf16),
# "f32" (everything f32; slowest, exact).
W_CFG = "bf16"

_built = None


def _dtypes():
    import ml_dtypes

    if W_CFG == "bf16":
        return ml_dtypes.bfloat16, ml_dtypes.bfloat16
    if W_CFG == "mixed":
        return np.float32, ml_dtypes.bfloat16
    return np.float32, np.float32


def _gate_cols(W, i):
    """Column slice of a [in, 3*H] GRU weight for hidden shard i, gates
    re-packed as [z(128) | r(128) | h(128)]."""
    return np.concatenate(
        [W[:, g * H + i * HS : g * H + (i + 1) * HS] for g in range(3)], axis=1
    )


def _build_kernel(gdt_np, hdt_np):
    from concourse import mybir, tile, bacc
    import concourse.bass as bass
    from concourse.bass import _add_dep_helper
    from concourse.masks import make_identity

    f32 = mybir.dt.float32
    gdt = mybir.dt.from_np(gdt_np)
    hdt = mybir.dt.from_np(hdt_np)
    head_resident = hdt != f32  # 8 bf16 ffw tiles fit in SBUF

    nc = bacc.Bacc(
        "TRN2", target_bir_lowering=False, debug=False, num_devices=NCORES
    )

    # ---- DRAM I/O ----
    x_ids = nc.dram_tensor("x_ids", [B, 1], mybir.dt.int32, kind="ExternalInput")
    emb = nc.dram_tensor("emb", [V, EMB], gdt, kind="ExternalInput")
    h0T_d = nc.dram_tensor("h0T", [H, B], gdt, kind="ExternalInput")
    h1T_d = nc.dram_tensor("h1T", [H, B], gdt, kind="ExternalInput")
    h0sl_d = nc.dram_tensor("h0sl", [B, HS], f32, kind="ExternalInput")
    h1sl_d = nc.dram_tensor("h1sl", [B, HS], f32, kind="ExternalInput")
    wi0_d = nc.dram_tensor("wi0", [EMB, 3 * HS], gdt, kind="ExternalInput")
    wr0_d = nc.dram_tensor("wr0", [H, 3 * HS], gdt, kind="ExternalInput")
    wi1_d = nc.dram_tensor("wi1", [H, 3 * HS], gdt, kind="ExternalInput")
    wr1_d = nc.dram_tensor("wr1", [H, 3 * HS], gdt, kind="ExternalInput")
    b0_d = nc.dram_tensor("b0", [1, 4 * HS], gdt, kind="ExternalInput")
    b1_d = nc.dram_tensor("b1", [1, 4 * HS], gdt, kind="ExternalInput")
    ffw_d = nc.dram_tensor("ffw", [H, VS], hdt, kind="ExternalInput")
    ffb_d = nc.dram_tensor("ffb", [1, VS], hdt, kind="ExternalInput")

    logits_o = nc.dram_tensor("logits_s", [B, VS], f32, kind="ExternalOutput")
    h0n_o = nc.dram_tensor("h0n_s", [B, HS], f32, kind="ExternalOutput")
    h1n_o = nc.dram_tensor("h1n_s", [B, HS], f32, kind="ExternalOutput")

    P = 128
    RG = [list(range(NCORES))]
    S = 3 * HS  # 384: packed z|r|h gate columns per k-block

    with tile.TileContext(nc) as tc:
        with (
            tc.tile_pool(name="const", bufs=1) as cpool,
            tc.tile_pool(name="wts", bufs=1) as wpool,
            tc.tile_pool(name="acts", bufs=2) as apool,
            tc.tile_pool(name="ffp", bufs=8 if head_resident else 6) as ffp,
            tc.tile_pool(name="outp", bufs=3) as opool,
            tc.tile_pool(name="dram", bufs=2, space="DRAM") as dpool,
        ):
            # ---- constants ----
            ident32 = cpool.tile([P, P], f32)
            make_identity(nc, ident32[:])
            if gdt != f32:
                identg = cpool.tile([P, P], gdt)
                make_identity(nc, identg[:])
            else:
                identg = ident32
            ones_g = cpool.tile([1, P], gdt)
            nc.gpsimd.memset(ones_g[:], 1.0)
            if hdt != gdt:
                ones_h = cpool.tile([1, P], hdt)
                nc.gpsimd.memset(ones_h[:], 1.0)
            else:
                ones_h = ones_g

            # ---- load indices and gather embedding rows (gpsimd/SWDGE) ----
            idx = cpool.tile([B, 1], mybir.dt.int32)
            nc.sync.dma_start(idx[:], x_ids[:])
            e_sb = cpool.tile([P, EMB], gdt)
            nc.gpsimd.indirect_dma_start(
                out=e_sb[:],
                out_offset=None,
                in_=emb[:],
                in_offset=bass.IndirectOffsetOnAxis(ap=idx[:, :1], axis=0),
            )

            # ---- GRU weights / state loads (SP ring; all resident) ----
            wi0_sb = wpool.tile([P, 2 * S], gdt)
            for k in range(2):
                nc.sync.dma_start(
                    wi0_sb[:, k * S : (k + 1) * S], wi0_d[k * P : (k + 1) * P, :]
                )
            wr0_sb = wpool.tile([P, 8 * S], gdt)
            wi1_sb = wpool.tile([P, 8 * S], gdt)
            wr1_sb = wpool.tile([P, 8 * S], gdt)
            for sb, d in ((wr0_sb, wr0_d), (wi1_sb, wi1_d), (wr1_sb, wr1_d)):
                for k in range(8):
                    nc.sync.dma_start(
                        sb[:, k * S : (k + 1) * S], d[k * P : (k + 1) * P, :]
                    )
            h0T_sb = wpool.tile([P, H], gdt)
            h1T_sb = wpool.tile([P, H], gdt)
            for sb, d in ((h0T_sb, h0T_d), (h1T_sb, h1T_d)):
                for k in range(8):
                    nc.sync.dma_start(
                        sb[:, k * P : (k + 1) * P], d[k * P : (k + 1) * P, :]
                    )
            h0sl = cpool.tile([B, HS], f32)
            nc.sync.dma_start(h0sl[:], h0sl_d[:])
            h1sl = cpool.tile([B, HS], f32)
            nc.sync.dma_start(h1sl[:], h1sl_d[:])
            b0_sb = cpool.tile([1, 4 * HS], gdt)
            nc.sync.dma_start(b0_sb[:], b0_d[:])
            b1_sb = cpool.tile([1, 4 * HS], gdt)
            nc.sync.dma_start(b1_sb[:], b1_d[:])
            ffb_sb = cpool.tile([1, VS], hdt)
            last_small = nc.sync.dma_start(ffb_sb[:], ffb_d[:])

            # ---- head weight stream ----
            # When resident (bf16 head): issue on the ACT ring, gated on the
            # GRU loads above so they win the HBM bandwidth race.
            fw_tiles = []
            if head_resident:
                for k in range(8):
                    fw = ffp.tile([P, VS], hdt, tag="fw", name=f"fw{k}")
                    d = nc.scalar.dma_start(fw[:], ffw_d[k * P : (k + 1) * P, :])
                    if k == 0:
                        _add_dep_helper(
                            d.ins, last_small.ins, sync=True,
                            reason="gru loads first",
                        )
                    fw_tiles.append(fw)

            with tc.tile_pool(name="psg", bufs=2, space="PSUM") as psg:
                # ---- transpose gathered embeddings: eT[k] = e[:, k*128:].T ----
                eT = cpool.tile([P, EMB], gdt)
                for k in range(2):
                    trw = psg.tile([P, P], gdt, tag="trw", bufs=1, name=f"trw{k}")
                    nc.tensor.transpose(
                        trw[:], e_sb[:, k * P : (k + 1) * P], identg[:]
                    )
                    nc.vector.tensor_copy(eT[:, k * P : (k + 1) * P], trw[:])

                def alloc_psums(lname):
                    zr = psg.tile([B, 2 * HS], f32, tag="zr", name=f"zr_{lname}")
                    ih = psg.tile([B, HS], f32, tag="ih", name=f"ih_{lname}")
                    rh = psg.tile([B, HS], f32, tag="rh", name=f"rh_{lname}")
                    return zr, ih, rh

                def bias_mms(ps, b_sb):
                    zr, ih, rh = ps
                    nc.tensor.matmul(
                        zr[:], ones_g[:1, :], b_sb[:1, 0 : 2 * HS],
                        start=True, stop=False,
                    )
                    nc.tensor.matmul(
                        ih[:], ones_g[:1, :], b_sb[:1, 2 * HS : 3 * HS],
                        start=True, stop=False,
                    )
                    nc.tensor.matmul(
                        rh[:], ones_g[:1, :], b_sb[:1, 3 * HS : 4 * HS],
                        start=True, stop=False,
                    )

                def x_side(ps, xT_sb, nkx, wi_sb, zr_stop=False):
                    zr, ih, rh = ps
                    for k in range(nkx):
                        xT = xT_sb[:, k * P : (k + 1) * P]
                        nc.tensor.matmul(
                            zr[:], xT, wi_sb[:, k * S : k * S + 2 * HS],
                            start=False, stop=(zr_stop and k == nkx - 1),
                        )
                        nc.tensor.matmul(
                            ih[:], xT, wi_sb[:, k * S + 2 * HS : (k + 1) * S],
                            start=False, stop=(k == nkx - 1),
                        )

                def h_side(ps, hT_sb, wr_sb, zr_stop):
                    zr, ih, rh = ps
                    for k in range(8):
                        hT = hT_sb[:, k * P : (k + 1) * P]
                        nc.tensor.matmul(
                            zr[:], hT, wr_sb[:, k * S : k * S + 2 * HS],
                            start=False, stop=(zr_stop and k == 7),
                        )
                        nc.tensor.matmul(
                            rh[:], hT, wr_sb[:, k * S + 2 * HS : (k + 1) * S],
                            start=False, stop=(k == 7),
                        )

                def gates(lname, ps, hsl, hn_out, out_dt):
                    zr, ih, rh = ps
                    z = apool.tile([B, HS], f32, tag="z", name=f"z_{lname}")
                    nc.scalar.activation(
                        z[:], zr[:, 0:HS], mybir.ActivationFunctionType.Sigmoid
                    )
                    r = apool.tile([B, HS], f32, tag="r", name=f"r_{lname}")
                    nc.scalar.activation(
                        r[:], zr[:, HS : 2 * HS],
                        mybir.ActivationFunctionType.Sigmoid,
                    )
                    t = apool.tile([B, HS], f32, tag="t", name=f"t_{lname}")
                    nc.vector.tensor_mul(t[:], r[:], rh[:])
                    u = apool.tile([B, HS], f32, tag="u", name=f"u_{lname}")
                    nc.vector.tensor_add(u[:], ih[:], t[:])
                    hh = apool.tile([B, HS], f32, tag="hh", name=f"hh_{lname}")
                    nc.scalar.activation(
                        hh[:], u[:], mybir.ActivationFunctionType.Tanh
                    )
                    d = apool.tile([B, HS], f32, tag="d", name=f"d_{lname}")
                    nc.vector.tensor_sub(d[:], hsl[:], hh[:])
                    zd = apool.tile([B, HS], f32, tag="zd", name=f"zd_{lname}")
                    nc.vector.tensor_mul(zd[:], z[:], d[:])
                    hn = apool.tile([B, HS], f32, tag="hn", name=f"hn_{lname}")
                    nc.vector.tensor_add(hn[:], hh[:], zd[:])
                    nc.sync.dma_start(hn_out[:], hn[:])
                    tr = psg.tile([P, HS], f32, tag="tr", bufs=1, name=f"tr_{lname}")
                    nc.tensor.transpose(tr[:], hn[:], ident32[:])
                    hnT = apool.tile([P, HS], out_dt, tag="hnT", name=f"hnT_{lname}")
                    nc.vector.tensor_copy(hnT[:], tr[:])
                    return hnT

                def ag_trigger(lname, hnT, dt):
                    agi = dpool.tile([P, HS], dt, tag=f"agi_{lname}", name=f"agi_{lname}")
                    nc.sync.dma_start(agi[:], hnT[:])
                    ago = dpool.tile(
                        [H, B], dt, tag=f"ago_{lname}", name=f"ago_{lname}",
                        addr_space="Shared",
                    )
                    nc.gpsimd.collective_compute(
                        "AllGather",
                        mybir.AluOpType.bypass,
                        replica_groups=RG,
                        ins=[agi[:].opt()],
                        outs=[ago[:].opt()],
                    )
                    return ago

                def ag_load(ago, dst_sb):
                    for k in range(8):
                        nc.sync.dma_start(
                            dst_sb[:, k * P : (k + 1) * P],
                            ago[k * P : (k + 1) * P, :],
                        )

                h0nT_sb = wpool.tile([P, H], gdt)
                h1nT_sb = wpool.tile([P, H], hdt)

                # ---- layer 0 ----
                ps0 = alloc_psums("l0")
                bias_mms(ps0, b0_sb)
                x_side(ps0, eT, 2, wi0_sb)
                h_side(ps0, h0T_sb, wr0_sb, zr_stop=True)
                hnT0 = gates("l0", ps0, h0sl, h0n_o, gdt)
                ago0 = ag_trigger("l0", hnT0, gdt)

                # ---- layer 1 h-side runs during AllGather 0 ----
                ps1 = alloc_psums("l1")
                bias_mms(ps1, b1_sb)
                h_side(ps1, h1T_sb, wr1_sb, zr_stop=False)
                ag_load(ago0, h0nT_sb)
                x_side(ps1, h0nT_sb, 8, wi1_sb, zr_stop=True)
                hnT1 = gates("l1", ps1, h1sl, h1n_o, hdt)
                ago1 = ag_trigger("l1", hnT1, hdt)
                ag_load(ago1, h1nT_sb)

            # ---- vocab head: logits_s = h1n @ ffW_s + ffb_s ----
            with tc.tile_pool(name="psh", bufs=1, space="PSUM") as psh:
                pls = [
                    psh.tile([B, NTS], f32, tag=f"pl{j}", name=f"pl{j}")
                    for j in range(NT)
                ]
                for j in range(NT):
                    nc.tensor.matmul(
                        pls[j][:], ones_h[:1, :],
                        ffb_sb[:1, j * NTS : (j + 1) * NTS],
                        start=True, stop=False,
                    )
                for k in range(8):
                    if head_resident:
                        fw = fw_tiles[k]
                    else:
                        fw = ffp.tile([P, VS], hdt, tag="fw", name=f"fw{k}")
                        nc.sync.dma_start(fw[:], ffw_d[k * P : (k + 1) * P, :])
                    h1nT_k = h1nT_sb[:, k * P : (k + 1) * P]
                    for j in range(NT):
                        nc.tensor.matmul(
                            pls[j][:], h1nT_k, fw[:, j * NTS : (j + 1) * NTS],
                            start=False, stop=(k == 7),
                        )
                for j in range(NT):
                    lo = opool.tile([B, NTS], f32, tag="lo", name=f"lo{j}")
                    nc.vector.tensor_copy(lo[:], pls[j][:])
                    nc.sync.dma_start(logits_o[:, j * NTS : (j + 1) * NTS], lo[:])

    nc.compile()
    return nc


def _make_runner(nc):
    """Build a jit'd 8-core executor with pre-placed device inputs.

    Mirrors concourse.bass2jax.run_bass_via_pjrt's multi-core path, but
    exposes input placement so host->device transfers are excluded from the
    measured NEFF execution (avoids cross-core launch skew from transfer
    completion times).
    """
    import jax
    from jax.sharding import Mesh, PartitionSpec, NamedSharding
    from jax.experimental.shard_map import shard_map
    from concourse import mybir
    import concourse.bass2jax as b2j

    b2j.install_neuronx_cc_hook()

    partition_name = (
        nc.partition_id_tensor.name if nc.partition_id_tensor else None
    )
    in_names, out_names, out_avals, zero_shapes = [], [], [], []
    for alloc in nc.m.functions[0].allocations:
        if not isinstance(alloc, mybir.MemoryLocationSet):
            continue
        name = alloc.memorylocations[0].name
        if alloc.kind == "ExternalInput":
            if name != partition_name:
                in_names.append(name)
        elif alloc.kind == "ExternalOutput":
            shape = tuple(alloc.tensor_shape)
            dtype = mybir.dt.np(alloc.dtype)
            out_names.append(name)
            out_avals.append(jax.core.ShapedArray(shape, dtype))
            zero_shapes.append((shape, dtype))
    n_params = len(in_names)
    n_outs = len(out_names)
    all_in_names = list(in_names) + list(out_names)
    if partition_name is not None:
        all_in_names.append(partition_name)

    donate = tuple(range(n_params, n_params + n_outs))

    def _body(*args):
        operands = list(args)
        if partition_name is not None:
            operands.append(b2j.partition_id_tensor())
        outs = b2j._bass_exec_p.bind(
            *operands,
            out_avals=tuple(out_avals),
            in_names=tuple(all_in_names),
            out_names=tuple(out_names),
            lowering_input_output_aliases=(),
            sim_require_finite=True,
            sim_require_nnan=True,
            nc=nc,
        )
        return tuple(outs)

    devices = jax.devices()[:NCORES]
    mesh = Mesh(np.asarray(devices), ("core",))
    in_specs = (PartitionSpec("core"),) * (n_params + n_outs)
    out_specs = (PartitionSpec("core"),) * n_outs
    sharded = jax.jit(
        shard_map(
            _body, mesh=mesh, in_specs=in_specs, out_specs=out_specs,
            check_rep=False,
        ),
        donate_argnums=donate,
        keep_unused=True,
    )
    sharding = NamedSharding(mesh, PartitionSpec("core"))

    def run(in_maps):
        import jax

        concat_in = [
            np.ascontiguousarray(
                np.concatenate(
                    [np.asarray(in_maps[c][n]) for c in range(NCORES)], axis=0
                )
            )
            for n in in_names
        ]
        dev_in = [jax.device_put(a, sharding) for a in concat_in]
        for a in dev_in:
            a.block_until_ready()

        def call():
            zeros = [
                jax.device_put(np.zeros((NCORES * s[0], *s[1:]), dt), sharding)
                for (s, dt) in zero_shapes
            ]
            for z in zeros:
                z.block_until_ready()
            outs = sharded(*dev_in, *zeros)
            for o in outs:
                o.block_until_ready()
            return outs

        call()  # warmup: compile + first NEFF execution
        outs = call()
        res = [
            {
                name: np.asarray(outs[i]).reshape(NCORES, *out_avals[i].shape)[c]
                for i, name in enumerate(out_names)
            }
            for c in range(NCORES)
        ]
        return res, call

    return run


def _get():
    global _built
    if _built is None:
        gdt_np, hdt_np = _dtypes()
        nc = _build_kernel(gdt_np, hdt_np)
        runner = _make_runner(nc)
        _built = (nc, runner, (gdt_np, hdt_np))
    return _built


def kernel(**inputs):
    nc, runner, dts = _get()
    in_maps = _prepare_in_maps(inputs, dts)
    res, _ = runner(in_maps)
    return _assemble(res)


def _prepare_in_maps(inputs, dts):
    gdt_np, hdt_np = dts
    f32 = np.float32
    x = np.asarray(inputs["x"]).astype(np.int32).reshape(B, 1)
    h0 = np.asarray(inputs["h0"], dtype=f32)
    h1 = np.asarray(inputs["h1"], dtype=f32)
    emb = np.asarray(inputs["emb"], dtype=f32)
    Wi0 = np.asarray(inputs["Wi0"], dtype=f32)
    Wr0 = np.asarray(inputs["Wr0"], dtype=f32)
    bi0 = np.asarray(inputs["bi0"], dtype=f32)
    br0 = np.asarray(inputs["br0"], dtype=f32)
    Wi1 = np.asarray(inputs["Wi1"], dtype=f32)
    Wr1 = np.asarray(inputs["Wr1"], dtype=f32)
    bi1 = np.asarray(inputs["bi1"], dtype=f32)
    br1 = np.asarray(inputs["br1"], dtype=f32)
    ffW = np.asarray(inputs["ffW"], dtype=f32)
    ffb = np.asarray(inputs["ffb"], dtype=f32)

    g = lambda a: np.ascontiguousarray(a).astype(gdt_np)
    hcast = lambda a: np.ascontiguousarray(a).astype(hdt_np)
    emb_g = g(emb)
    h0T = g(h0.T)
    h1T = g(h1.T)

    def layer_bias(bi, br, i):
        bsum = bi + br
        bz = bsum[0 * H + i * HS : 0 * H + (i + 1) * HS]
        br_ = bsum[1 * H + i * HS : 1 * H + (i + 1) * HS]
        bih = bi[2 * H + i * HS : 2 * H + (i + 1) * HS]
        brh = br[2 * H + i * HS : 2 * H + (i + 1) * HS]
        return g(np.concatenate([bz, br_, bih, brh])[None, :])

    in_maps = []
    for i in range(NCORES):
        in_maps.append(
            {
                "x_ids": x,
                "emb": emb_g,
                "h0T": h0T,
                "h1T": h1T,
                "h0sl": np.ascontiguousarray(h0[:, i * HS : (i + 1) * HS]),
                "h1sl": np.ascontiguousarray(h1[:, i * HS : (i + 1) * HS]),
                "wi0": g(_gate_cols(Wi0, i)),
                "wr0": g(_gate_cols(Wr0, i)),
                "wi1": g(_gate_cols(Wi1, i)),
                "wr1": g(_gate_cols(Wr1, i)),
                "b0": layer_bias(bi0, br0, i),
                "b1": layer_bias(bi1, br1, i),
                "ffw": hcast(ffW[:, i * VS : (i + 1) * VS]),
                "ffb": hcast(ffb[i * VS : (i + 1) * VS][None, :]),
            }
        )
    return in_maps


def _assemble(res):
    logits = np.concatenate([res[c]["logits_s"] for c in range(NCORES)], axis=1)
    h0n = np.concatenate([res[c]["h0n_s"] for c in range(NCORES)], axis=1)
    h1n = np.concatenate([res[c]["h1n_s"] for c in range(NCORES)], axis=1)
    return logits, h0n, h1n


# revision 7
# speedup vs baseline: 1.5978x; 1.1716x over previous
"""Trainium2 Bass kernel: 2-layer GRU decoder step with a 32k-vocab head.

Sharding (8 NeuronCores, one chip):
  * GRU layers are tensor-parallel over the hidden dim: core i owns hidden
    units [i*128, (i+1)*128) of each layer, holding the matching column
    slices of Wi/Wr (gates z/r/h re-packed side by side). After each layer
    the per-core transposed state slices are AllGathered so the next stage
    has the full hidden state as a stationary operand.
  * The vocab head is tensor-parallel over the vocab dim: core i owns
    ffW[:, i*4000:(i+1)*4000] and produces logits[:, i*4000:(i+1)*4000].
  * The embedding gather runs on-device via indirect DMA (data-dependent).
Host side does layout-only staging: slicing, transposes, dtype casts, and
packing weights into SBUF-layout so each load is one large DMA.
"""

import numpy as np

B = 128
EMB = 256
H = 1024
V = 32000
NCORES = 8
HS = H // NCORES          # 128 hidden units per core per layer
VS = V // NCORES          # 4000 vocab columns per core
NT = 8                    # head column tiles per core (4000 = 8*500)
NTS = VS // NT            # 500
P = 128
S = 3 * HS                # 384 packed z|r|h gate columns per k-block

# W_CFG: "bf16" (GRU + head weights bf16) or "mixed" (GRU f32, head bf16).
W_CFG = "bf16"

_built = None


def _dtypes():
    import ml_dtypes

    if W_CFG == "bf16":
        return ml_dtypes.bfloat16, ml_dtypes.bfloat16
    return np.float32, ml_dtypes.bfloat16


def _gate_cols(W, i):
    """Column slice of a [in, 3*H] GRU weight for hidden shard i, gates
    re-packed as [z(128) | r(128) | h(128)]."""
    return np.concatenate(
        [W[:, g * H + i * HS : g * H + (i + 1) * HS] for g in range(3)], axis=1
    )


def _pack_w(W):
    """[K, n] -> [128, (K/128)*n] so sbuf[p, k*n+j] = W[k*128+p, j]."""
    K, n = W.shape
    return np.ascontiguousarray(
        W.reshape(K // P, P, n).transpose(1, 0, 2).reshape(P, (K // P) * n)
    )


def _pack_hT(h):
    """[B, H] -> [128, H] transposed-packed: sbuf[p, k*B+b] = h[b, k*128+p]."""
    return np.ascontiguousarray(
        h.reshape(B, H // P, P).transpose(2, 1, 0).reshape(P, H)
    )


def _build_kernel(gdt_np, hdt_np):
    from concourse import mybir, tile, bacc
    import concourse.bass as bass
    from concourse.masks import make_identity

    f32 = mybir.dt.float32
    gdt = mybir.dt.from_np(gdt_np)
    hdt = mybir.dt.from_np(hdt_np)

    nc = bacc.Bacc(
        "TRN2", target_bir_lowering=False, debug=False, num_devices=NCORES
    )

    # ---- DRAM I/O ----
    x_ids = nc.dram_tensor("x_ids", [B, 1], mybir.dt.int32, kind="ExternalInput")
    emb = nc.dram_tensor("emb", [V, EMB], gdt, kind="ExternalInput")
    # gw0: wi0|wr0 packed [128, 10*S]; gw1: wi1|wr1 packed [128, 16*S]
    gw0_d = nc.dram_tensor("gw0", [P, 10 * S], gdt, kind="ExternalInput")
    gw1_d = nc.dram_tensor("gw1", [P, 16 * S], gdt, kind="ExternalInput")
    hT2_d = nc.dram_tensor("hT2", [P, 2 * H], gdt, kind="ExternalInput")
    hsl2_d = nc.dram_tensor("hsl2", [B, 2 * HS], f32, kind="ExternalInput")
    b01_d = nc.dram_tensor("b01", [1, 8 * HS], gdt, kind="ExternalInput")
    ffb_d = nc.dram_tensor("ffb", [1, VS], hdt, kind="ExternalInput")
    fwp_d = nc.dram_tensor("fwp", [P, 8 * VS], hdt, kind="ExternalInput")

    logits_o = nc.dram_tensor("logits_s", [B, VS], f32, kind="ExternalOutput")
    h0n_o = nc.dram_tensor("h0n_s", [B, HS], f32, kind="ExternalOutput")
    h1n_o = nc.dram_tensor("h1n_s", [B, HS], f32, kind="ExternalOutput")

    RG = [list(range(NCORES))]

    with tile.TileContext(nc) as tc:
        with (
            tc.tile_pool(name="const", bufs=1) as cpool,
            tc.tile_pool(name="wts", bufs=1) as wpool,
            tc.tile_pool(name="acts", bufs=2) as apool,
            tc.tile_pool(name="outp", bufs=3) as opool,
            tc.tile_pool(name="dram", bufs=2, space="DRAM") as dpool,
        ):
            # ---- dummy AllGather: absorbs collective-stream warmup and
            # cross-core launch skew while local work proceeds ----
            dum = cpool.tile([1, 32], gdt)
            nc.gpsimd.memset(dum[:], 0.0)
            dum_in = dpool.tile([1, 32], gdt, tag="dum_in")
            nc.gpsimd.dma_start(dum_in[:], dum[:])
            dum_out = dpool.tile([NCORES, 32], gdt, tag="dum_out", addr_space="Shared")
            nc.gpsimd.collective_compute(
                "AllGather",
                mybir.AluOpType.bypass,
                replica_groups=RG,
                ins=[dum_in[:].opt()],
                outs=[dum_out[:].opt()],
            )

            # ---- constants ----
            ident32 = cpool.tile([P, P], f32)
            make_identity(nc, ident32[:])
            if gdt != f32:
                identg = cpool.tile([P, P], gdt)
                make_identity(nc, identg[:])
            else:
                identg = ident32
            ones_g = cpool.tile([1, P], gdt)
            nc.gpsimd.memset(ones_g[:], 1.0)
            if hdt != gdt:
                ones_h = cpool.tile([1, P], hdt)
                nc.gpsimd.memset(ones_h[:], 1.0)
            else:
                ones_h = ones_g

            # ---- SP-ring loads; FIFO order = bandwidth priority ----
            idx = cpool.tile([B, 1], mybir.dt.int32)
            nc.sync.dma_start(idx[:], x_ids[:])
            hsl2 = cpool.tile([B, 2 * HS], f32)
            nc.sync.dma_start(hsl2[:], hsl2_d[:])
            b01 = cpool.tile([1, 8 * HS], gdt)
            nc.sync.dma_start(b01[:], b01_d[:])
            gw0 = wpool.tile([P, 10 * S], gdt)
            nc.sync.dma_start(gw0[:], gw0_d[:])
            hT2 = wpool.tile([P, 2 * H], gdt)
            nc.sync.dma_start(hT2[:], hT2_d[:])
            gw1 = wpool.tile([P, 16 * S], gdt)
            nc.sync.dma_start(gw1[:], gw1_d[:])
            ffb_sb = cpool.tile([1, VS], hdt)
            nc.sync.dma_start(ffb_sb[:], ffb_d[:])
            fwp = wpool.tile([P, 8 * VS], hdt)
            nc.sync.dma_start(fwp[:], fwp_d[:])

            wi0_sb = gw0[:, 0 : 2 * S]
            wr0_sb = gw0[:, 2 * S : 10 * S]
            wi1_sb = gw1[:, 0 : 8 * S]
            wr1_sb = gw1[:, 8 * S : 16 * S]
            h0T_sb = hT2[:, 0:H]
            h1T_sb = hT2[:, H : 2 * H]
            h0sl = hsl2[:, 0:HS]
            h1sl = hsl2[:, HS : 2 * HS]
            b0_sb = b01[:, 0 : 4 * HS]
            b1_sb = b01[:, 4 * HS : 8 * HS]

            # ---- embedding gather (gpsimd/SWDGE, parallel ring) ----
            e_sb = cpool.tile([P, EMB], gdt)
            nc.gpsimd.indirect_dma_start(
                out=e_sb[:],
                out_offset=None,
                in_=emb[:],
                in_offset=bass.IndirectOffsetOnAxis(ap=idx[:, :1], axis=0),
            )

            with tc.tile_pool(name="psg", bufs=2, space="PSUM") as psg:
                # ---- transpose gathered embeddings ----
                eT = cpool.tile([P, EMB], gdt)
                for k in range(2):
                    trw = psg.tile([P, P], gdt, tag="trw", bufs=1, name=f"trw{k}")
                    nc.tensor.transpose(
                        trw[:], e_sb[:, k * P : (k + 1) * P], identg[:]
                    )
                    nc.vector.tensor_copy(eT[:, k * P : (k + 1) * P], trw[:])

                def alloc_psums(lname):
                    zr = psg.tile([B, 2 * HS], f32, tag="zr", name=f"zr_{lname}")
                    ih = psg.tile([B, HS], f32, tag="ih", name=f"ih_{lname}")
                    rh = psg.tile([B, HS], f32, tag="rh", name=f"rh_{lname}")
                    return zr, ih, rh

                def bias_mms(ps, b_sb):
                    zr, ih, rh = ps
                    nc.tensor.matmul(
                        zr[:], ones_g[:1, :], b_sb[:1, 0 : 2 * HS],
                        start=True, stop=False,
                    )
                    nc.tensor.matmul(
                        ih[:], ones_g[:1, :], b_sb[:1, 2 * HS : 3 * HS],
                        start=True, stop=False,
                    )
                    nc.tensor.matmul(
                        rh[:], ones_g[:1, :], b_sb[:1, 3 * HS : 4 * HS],
                        start=True, stop=False,
                    )

                def h_side(ps, hT_sb, wr_sb):
                    zr, ih, rh = ps
                    for k in range(8):
                        hT = hT_sb[:, k * P : (k + 1) * P]
                        nc.tensor.matmul(
                            zr[:], hT, wr_sb[:, k * S : k * S + 2 * HS],
                            start=False, stop=False,
                        )
                        nc.tensor.matmul(
                            rh[:], hT, wr_sb[:, k * S + 2 * HS : (k + 1) * S],
                            start=False, stop=(k == 7),
                        )

                def x_side(ps, xT_sb, nkx, wi_sb):
                    zr, ih, rh = ps
                    for k in range(nkx):
                        xT = xT_sb[:, k * P : (k + 1) * P]
                        nc.tensor.matmul(
                            zr[:], xT, wi_sb[:, k * S : k * S + 2 * HS],
                            start=False, stop=(k == nkx - 1),
                        )
                        nc.tensor.matmul(
                            ih[:], xT, wi_sb[:, k * S + 2 * HS : (k + 1) * S],
                            start=False, stop=(k == nkx - 1),
                        )

                def gates(lname, ps, hsl, hn_out, out_dt):
                    zr, ih, rh = ps
                    z = apool.tile([B, HS], f32, tag="z", name=f"z_{lname}")
                    nc.scalar.activation(
                        z[:], zr[:, 0:HS], mybir.ActivationFunctionType.Sigmoid
                    )
                    r = apool.tile([B, HS], f32, tag="r", name=f"r_{lname}")
                    nc.scalar.activation(
                        r[:], zr[:, HS : 2 * HS],
                        mybir.ActivationFunctionType.Sigmoid,
                    )
                    t = apool.tile([B, HS], f32, tag="t", name=f"t_{lname}")
                    nc.vector.tensor_mul(t[:], r[:], rh[:])
                    u = apool.tile([B, HS], f32, tag="u", name=f"u_{lname}")
                    nc.vector.tensor_add(u[:], ih[:], t[:])
                    hh = apool.tile([B, HS], f32, tag="hh", name=f"hh_{lname}")
                    nc.scalar.activation(
                        hh[:], u[:], mybir.ActivationFunctionType.Tanh
                    )
                    d = apool.tile([B, HS], f32, tag="d", name=f"d_{lname}")
                    nc.vector.tensor_sub(d[:], hsl[:], hh[:])
                    zd = apool.tile([B, HS], f32, tag="zd", name=f"zd_{lname}")
                    nc.vector.tensor_mul(zd[:], z[:], d[:])
                    hn = apool.tile([B, HS], f32, tag="hn", name=f"hn_{lname}")
                    nc.vector.tensor_add(hn[:], hh[:], zd[:])
                    nc.sync.dma_start(hn_out[:], hn[:])
                    tr = psg.tile([P, HS], f32, tag="tr", bufs=1, name=f"tr_{lname}")
                    nc.tensor.transpose(tr[:], hn[:], ident32[:])
                    hnT = apool.tile([P, HS], out_dt, tag="hnT", name=f"hnT_{lname}")
                    nc.vector.tensor_copy(hnT[:], tr[:])
                    return hnT

                def ag_trigger(lname, hnT, dt):
                    agi = dpool.tile(
                        [P, HS], dt, tag=f"agi_{lname}", name=f"agi_{lname}"
                    )
                    nc.sync.dma_start(agi[:], hnT[:])
                    ago = dpool.tile(
                        [H, B], dt, tag=f"ago_{lname}", name=f"ago_{lname}",
                        addr_space="Shared",
                    )
                    nc.gpsimd.collective_compute(
                        "AllGather",
                        mybir.AluOpType.bypass,
                        replica_groups=RG,
                        ins=[agi[:].opt()],
                        outs=[ago[:].opt()],
                    )
                    return ago

                def ag_load(ago, dst_sb):
                    # one DMA: sbuf[p, k*B+b] = ago[k*128+p, b]
                    nc.sync.dma_start(
                        dst_sb.rearrange("p (k b) -> p k b", k=8),
                        ago[:].rearrange("(k p) b -> p k b", p=P),
                    )

                h0nT_sb = wpool.tile([P, H], gdt)
                h1nT_sb = wpool.tile([P, H], hdt)

                # ---- layer 0 (h-side first: eT arrives after the gather) ----
                ps0 = alloc_psums("l0")
                bias_mms(ps0, b0_sb)
                h_side(ps0, h0T_sb, wr0_sb)
                x_side(ps0, eT, 2, wi0_sb)
                hnT0 = gates("l0", ps0, h0sl, h0n_o, gdt)
                ago0 = ag_trigger("l0", hnT0, gdt)

                # ---- layer 1 h-side runs during AllGather 0 ----
                ps1 = alloc_psums("l1")
                bias_mms(ps1, b1_sb)
                h_side(ps1, h1T_sb, wr1_sb)
                ag_load(ago0, h0nT_sb[:])
                x_side(ps1, h0nT_sb, 8, wi1_sb)
                hnT1 = gates("l1", ps1, h1sl, h1n_o, hdt)
                ago1 = ag_trigger("l1", hnT1, hdt)
                ag_load(ago1, h1nT_sb[:])

            # ---- vocab head: logits_s = h1n @ ffW_s + ffb_s (j-outer) ----
            with tc.tile_pool(name="psh", bufs=3, space="PSUM") as psh:
                for j in range(NT):
                    pl = psh.tile([B, NTS], f32, tag="pl", name=f"pl{j}")
                    nc.tensor.matmul(
                        pl[:], ones_h[:1, :], ffb_sb[:1, j * NTS : (j + 1) * NTS],
                        start=True, stop=False,
                    )
                    for k in range(8):
                        nc.tensor.matmul(
                            pl[:],
                            h1nT_sb[:, k * P : (k + 1) * P],
                            fwp[:, k * VS + j * NTS : k * VS + (j + 1) * NTS],
                            start=False, stop=(k == 7),
                        )
                    lo = opool.tile([B, NTS], f32, tag="lo", name=f"lo{j}")
                    nc.vector.tensor_copy(lo[:], pl[:])
                    nc.sync.dma_start(logits_o[:, j * NTS : (j + 1) * NTS], lo[:])

    nc.compile()
    return nc


def _make_runner(nc):
    """Build a jit'd 8-core executor with pre-placed device inputs.

    Mirrors concourse.bass2jax.run_bass_via_pjrt's multi-core path, but
    exposes input placement so host->device transfers are excluded from the
    measured NEFF execution (avoids cross-core launch skew from transfer
    completion times).
    """
    import jax
    from jax.sharding import Mesh, PartitionSpec, NamedSharding
    from jax.experimental.shard_map import shard_map
    from concourse import mybir
    import concourse.bass2jax as b2j

    b2j.install_neuronx_cc_hook()

    partition_name = (
        nc.partition_id_tensor.name if nc.partition_id_tensor else None
    )
    in_names, out_names, out_avals, zero_shapes = [], [], [], []
    for alloc in nc.m.functions[0].allocations:
        if not isinstance(alloc, mybir.MemoryLocationSet):
            continue
        name = alloc.memorylocations[0].name
        if alloc.kind == "ExternalInput":
            if name != partition_name:
                in_names.append(name)
        elif alloc.kind == "ExternalOutput":
            shape = tuple(alloc.tensor_shape)
            dtype = mybir.dt.np(alloc.dtype)
            out_names.append(name)
            out_avals.append(jax.core.ShapedArray(shape, dtype))
            zero_shapes.append((shape, dtype))
    n_params = len(in_names)
    n_outs = len(out_names)
    all_in_names = list(in_names) + list(out_names)
    if partition_name is not None:
        all_in_names.append(partition_name)

    donate = tuple(range(n_params, n_params + n_outs))

    def _body(*args):
        operands = list(args)
        if partition_name is not None:
            operands.append(b2j.partition_id_tensor())
        outs = b2j._bass_exec_p.bind(
            *operands,
            out_avals=tuple(out_avals),
            in_names=tuple(all_in_names),
            out_names=tuple(out_names),
            lowering_input_output_aliases=(),
            sim_require_finite=True,
            sim_require_nnan=True,
            nc=nc,
        )
        return tuple(outs)

    devices = jax.devices()[:NCORES]
    mesh = Mesh(np.asarray(devices), ("core",))
    in_specs = (PartitionSpec("core"),) * (n_params + n_outs)
    out_specs = (PartitionSpec("core"),) * n_outs
    sharded = jax.jit(
        shard_map(
            _body, mesh=mesh, in_specs=in_specs, out_specs=out_specs,
            check_rep=False,
        ),
        donate_argnums=donate,
        keep_unused=True,
    )
    sharding = NamedSharding(mesh, PartitionSpec("core"))

    def run(in_maps):
        import jax

        concat_in = [
            np.ascontiguousarray(
                np.concatenate(
                    [np.asarray(in_maps[c][n]) for c in range(NCORES)], axis=0
                )
            )
            for n in in_names
        ]
        dev_in = [jax.device_put(a, sharding) for a in concat_in]
        for a in dev_in:
            a.block_until_ready()

        def call():
            zeros = [
                jax.device_put(np.zeros((NCORES * s[0], *s[1:]), dt), sharding)
                for (s, dt) in zero_shapes
            ]
            for z in zeros:
                z.block_until_ready()
            outs = sharded(*dev_in, *zeros)
            for o in outs:
                o.block_until_ready()
            return outs

        call()  # warmup: compile + first NEFF execution
        outs = call()
        res = [
            {
                name: np.asarray(outs[i]).reshape(NCORES, *out_avals[i].shape)[c]
                for i, name in enumerate(out_names)
            }
            for c in range(NCORES)
        ]
        return res, call

    return run


def _get():
    global _built
    if _built is None:
        gdt_np, hdt_np = _dtypes()
        nc = _build_kernel(gdt_np, hdt_np)
        runner = _make_runner(nc)
        _built = (nc, runner, (gdt_np, hdt_np))
    return _built


def kernel(**inputs):
    nc, runner, dts = _get()
    in_maps = _prepare_in_maps(inputs, dts)
    res, _ = runner(in_maps)
    return _assemble(res)


def _prepare_in_maps(inputs, dts):
    gdt_np, hdt_np = dts
    f32 = np.float32
    x = np.asarray(inputs["x"]).astype(np.int32).reshape(B, 1)
    h0 = np.asarray(inputs["h0"], dtype=f32)
    h1 = np.asarray(inputs["h1"], dtype=f32)
    emb = np.asarray(inputs["emb"], dtype=f32)
    Wi0 = np.asarray(inputs["Wi0"], dtype=f32)
    Wr0 = np.asarray(inputs["Wr0"], dtype=f32)
    bi0 = np.asarray(inputs["bi0"], dtype=f32)
    br0 = np.asarray(inputs["br0"], dtype=f32)
    Wi1 = np.asarray(inputs["Wi1"], dtype=f32)
    Wr1 = np.asarray(inputs["Wr1"], dtype=f32)
    bi1 = np.asarray(inputs["bi1"], dtype=f32)
    br1 = np.asarray(inputs["br1"], dtype=f32)
    ffW = np.asarray(inputs["ffW"], dtype=f32)
    ffb = np.asarray(inputs["ffb"], dtype=f32)

    g = lambda a: np.ascontiguousarray(a).astype(gdt_np)
    hc = lambda a: np.ascontiguousarray(a).astype(hdt_np)
    emb_g = g(emb)
    hT2 = g(np.concatenate([_pack_hT(h0), _pack_hT(h1)], axis=1))

    def layer_bias(bi, br, i):
        bsum = bi + br
        bz = bsum[0 * H + i * HS : 0 * H + (i + 1) * HS]
        br_ = bsum[1 * H + i * HS : 1 * H + (i + 1) * HS]
        bih = bi[2 * H + i * HS : 2 * H + (i + 1) * HS]
        brh = br[2 * H + i * HS : 2 * H + (i + 1) * HS]
        return np.concatenate([bz, br_, bih, brh])

    in_maps = []
    for i in range(NCORES):
        gw0 = np.concatenate(
            [_pack_w(_gate_cols(Wi0, i)), _pack_w(_gate_cols(Wr0, i))], axis=1
        )
        gw1 = np.concatenate(
            [_pack_w(_gate_cols(Wi1, i)), _pack_w(_gate_cols(Wr1, i))], axis=1
        )
        b01 = np.concatenate(
            [layer_bias(bi0, br0, i), layer_bias(bi1, br1, i)]
        )[None, :]
        in_maps.append(
            {
                "x_ids": x,
                "emb": emb_g,
                "gw0": g(gw0),
                "gw1": g(gw1),
                "hT2": hT2,
                "hsl2": np.ascontiguousarray(
                    np.concatenate(
                        [
                            h0[:, i * HS : (i + 1) * HS],
                            h1[:, i * HS : (i + 1) * HS],
                        ],
                        axis=1,
                    )
                ),
                "b01": g(b01),
                "ffb": hc(ffb[i * VS : (i + 1) * VS][None, :]),
                "fwp": hc(_pack_w(ffW[:, i * VS : (i + 1) * VS])),
            }
        )
    return in_maps


def _assemble(res):
    logits = np.concatenate([res[c]["logits_s"] for c in range(NCORES)], axis=1)
    h0n = np.concatenate([res[c]["h0n_s"] for c in range(NCORES)], axis=1)
    h1n = np.concatenate([res[c]["h1n_s"] for c in range(NCORES)], axis=1)
    return logits, h0n, h1n


# revision 9
# speedup vs baseline: 1.6812x; 1.0522x over previous
"""Trainium2 Bass kernel: 2-layer GRU decoder step with a 32k-vocab head.

Sharding (8 NeuronCores, one chip):
  * GRU layers are tensor-parallel over the hidden dim: core i owns hidden
    units [i*128, (i+1)*128) of each layer, holding the matching column
    slices of Wi/Wr (gates z/r/h re-packed side by side). After each layer
    the per-core transposed state slices are AllGathered so the next stage
    has the full hidden state as a stationary operand.
  * The vocab head is tensor-parallel over the vocab dim: core i owns
    ffW[:, i*4000:(i+1)*4000] and produces logits[:, i*4000:(i+1)*4000].
  * The embedding gather runs on-device via indirect DMA (data-dependent).
Host side does layout-only staging: slicing, transposes, dtype casts, and
packing weights into SBUF-layout so each load is one large DMA.
"""

import numpy as np

B = 128
EMB = 256
H = 1024
V = 32000
NCORES = 8
HS = H // NCORES          # 128 hidden units per core per layer
VS = V // NCORES          # 4000 vocab columns per core
NT = 8                    # head column tiles per core (4000 = 8*500)
NTS = VS // NT            # 500
P = 128
S = 3 * HS                # 384 packed z|r|h gate columns per k-block

# W_CFG: "bf16" (GRU + head weights bf16) or "mixed" (GRU f32, head bf16).
W_CFG = "bf16"

_built = None


def _dtypes():
    import ml_dtypes

    if W_CFG == "bf16":
        return ml_dtypes.bfloat16, ml_dtypes.bfloat16
    return np.float32, ml_dtypes.bfloat16


def _gate_cols(W, i):
    """Column slice of a [in, 3*H] GRU weight for hidden shard i, gates
    re-packed as [z(128) | r(128) | h(128)]."""
    return np.concatenate(
        [W[:, g * H + i * HS : g * H + (i + 1) * HS] for g in range(3)], axis=1
    )


def _pack_w(W):
    """[K, n] -> [128, (K/128)*n] so sbuf[p, k*n+j] = W[k*128+p, j]."""
    K, n = W.shape
    return np.ascontiguousarray(
        W.reshape(K // P, P, n).transpose(1, 0, 2).reshape(P, (K // P) * n)
    )


def _pack_hT(h):
    """[B, H] -> [128, H] transposed-packed: sbuf[p, k*B+b] = h[b, k*128+p]."""
    return np.ascontiguousarray(
        h.reshape(B, H // P, P).transpose(2, 1, 0).reshape(P, H)
    )


def _build_kernel(gdt_np, hdt_np):
    from concourse import mybir, tile, bacc
    import concourse.bass as bass
    from concourse.masks import make_identity

    f32 = mybir.dt.float32
    gdt = mybir.dt.from_np(gdt_np)
    hdt = mybir.dt.from_np(hdt_np)

    nc = bacc.Bacc(
        "TRN2", target_bir_lowering=False, debug=False, num_devices=NCORES
    )

    # ---- DRAM I/O ----
    x_ids = nc.dram_tensor("x_ids", [B, 1], mybir.dt.int32, kind="ExternalInput")
    emb = nc.dram_tensor("emb", [V, EMB], gdt, kind="ExternalInput")
    # gw0: wi0|wr0 packed [128, 10*S]; gw1: wi1|wr1 packed [128, 16*S]
    gw0_d = nc.dram_tensor("gw0", [P, 10 * S], gdt, kind="ExternalInput")
    gw1_d = nc.dram_tensor("gw1", [P, 16 * S], gdt, kind="ExternalInput")
    hT2_d = nc.dram_tensor("hT2", [P, 2 * H], gdt, kind="ExternalInput")
    hsl2_d = nc.dram_tensor("hsl2", [B, 2 * HS], f32, kind="ExternalInput")
    b01_d = nc.dram_tensor("b01", [1, 8 * HS], gdt, kind="ExternalInput")
    ffb_d = nc.dram_tensor("ffb", [1, VS], hdt, kind="ExternalInput")
    fwp_d = nc.dram_tensor("fwp", [P, 8 * VS], hdt, kind="ExternalInput")

    logits_o = nc.dram_tensor("logits_s", [B, VS], f32, kind="ExternalOutput")
    h0n_o = nc.dram_tensor("h0n_s", [B, HS], f32, kind="ExternalOutput")
    h1n_o = nc.dram_tensor("h1n_s", [B, HS], f32, kind="ExternalOutput")

    RG = [list(range(NCORES))]

    with tile.TileContext(nc) as tc:
        with (
            tc.tile_pool(name="const", bufs=1) as cpool,
            tc.tile_pool(name="wts", bufs=1) as wpool,
            tc.tile_pool(name="acts", bufs=2) as apool,
            tc.tile_pool(name="outp", bufs=3) as opool,
            tc.tile_pool(name="dram", bufs=2, space="DRAM") as dpool,
        ):
            # ---- dummy AllGather: absorbs collective-stream warmup and
            # cross-core launch skew while local work proceeds ----
            dum = cpool.tile([1, 32], gdt)
            nc.gpsimd.memset(dum[:], 0.0)
            dum_in = dpool.tile([1, 32], gdt, tag="dum_in")
            nc.gpsimd.dma_start(dum_in[:], dum[:])
            dum_out = dpool.tile([NCORES, 32], gdt, tag="dum_out", addr_space="Shared")
            nc.gpsimd.collective_compute(
                "AllGather",
                mybir.AluOpType.bypass,
                replica_groups=RG,
                ins=[dum_in[:].opt()],
                outs=[dum_out[:].opt()],
            )

            # ---- constants ----
            ident32 = cpool.tile([P, P], f32)
            make_identity(nc, ident32[:])
            if gdt != f32:
                identg = cpool.tile([P, P], gdt)
                make_identity(nc, identg[:])
            else:
                identg = ident32
            ones_g = cpool.tile([1, P], gdt)
            nc.gpsimd.memset(ones_g[:], 1.0)
            if hdt != gdt:
                ones_h = cpool.tile([1, P], hdt)
                nc.gpsimd.memset(ones_h[:], 1.0)
            else:
                ones_h = ones_g

            # ---- SP-ring loads; FIFO order = bandwidth priority ----
            idx = cpool.tile([B, 1], mybir.dt.int32)
            nc.sync.dma_start(idx[:], x_ids[:])
            hsl2 = cpool.tile([B, 2 * HS], f32)
            nc.sync.dma_start(hsl2[:], hsl2_d[:])
            b01 = cpool.tile([1, 8 * HS], gdt)
            nc.sync.dma_start(b01[:], b01_d[:])
            gw0 = wpool.tile([P, 10 * S], gdt)
            nc.sync.dma_start(gw0[:], gw0_d[:])
            hT2 = wpool.tile([P, 2 * H], gdt)
            nc.sync.dma_start(hT2[:], hT2_d[:])
            gw1 = wpool.tile([P, 16 * S], gdt)
            nc.sync.dma_start(gw1[:], gw1_d[:])
            ffb_sb = cpool.tile([1, VS], hdt)
            nc.sync.dma_start(ffb_sb[:], ffb_d[:])
            fwp = wpool.tile([P, 8 * VS], hdt)
            nc.sync.dma_start(fwp[:], fwp_d[:])

            wi0_sb = gw0[:, 0 : 2 * S]
            wr0_sb = gw0[:, 2 * S : 10 * S]
            wi1_sb = gw1[:, 0 : 8 * S]
            wr1_sb = gw1[:, 8 * S : 16 * S]
            h0T_sb = hT2[:, 0:H]
            h1T_sb = hT2[:, H : 2 * H]
            h0sl = hsl2[:, 0:HS]
            h1sl = hsl2[:, HS : 2 * HS]
            b0_sb = b01[:, 0 : 4 * HS]
            b1_sb = b01[:, 4 * HS : 8 * HS]

            # ---- embedding gather (gpsimd/SWDGE, parallel ring) ----
            e_sb = cpool.tile([P, EMB], gdt)
            nc.gpsimd.indirect_dma_start(
                out=e_sb[:],
                out_offset=None,
                in_=emb[:],
                in_offset=bass.IndirectOffsetOnAxis(ap=idx[:, :1], axis=0),
            )

            with tc.tile_pool(name="psg", bufs=2, space="PSUM") as psg:
                # ---- transpose gathered embeddings ----
                eT = cpool.tile([P, EMB], gdt)
                for k in range(2):
                    trw = psg.tile([P, P], gdt, tag="trw", bufs=1, name=f"trw{k}")
                    nc.tensor.transpose(
                        trw[:], e_sb[:, k * P : (k + 1) * P], identg[:]
                    )
                    nc.vector.tensor_copy(eT[:, k * P : (k + 1) * P], trw[:])

                def alloc_psums(lname):
                    zr = psg.tile([B, 2 * HS], f32, tag="zr", name=f"zr_{lname}")
                    ih = psg.tile([B, HS], f32, tag="ih", name=f"ih_{lname}")
                    rh = psg.tile([B, HS], f32, tag="rh", name=f"rh_{lname}")
                    return zr, ih, rh

                def bias_mms(ps, b_sb):
                    zr, ih, rh = ps
                    nc.tensor.matmul(
                        zr[:], ones_g[:1, :], b_sb[:1, 0 : 2 * HS],
                        start=True, stop=False,
                    )
                    nc.tensor.matmul(
                        ih[:], ones_g[:1, :], b_sb[:1, 2 * HS : 3 * HS],
                        start=True, stop=False,
                    )
                    nc.tensor.matmul(
                        rh[:], ones_g[:1, :], b_sb[:1, 3 * HS : 4 * HS],
                        start=True, stop=False,
                    )

                def h_side(ps, hT_sb, wr_sb):
                    zr, ih, rh = ps
                    for k in range(8):
                        hT = hT_sb[:, k * P : (k + 1) * P]
                        nc.tensor.matmul(
                            zr[:], hT, wr_sb[:, k * S : k * S + 2 * HS],
                            start=False, stop=False,
                        )
                        nc.tensor.matmul(
                            rh[:], hT, wr_sb[:, k * S + 2 * HS : (k + 1) * S],
                            start=False, stop=(k == 7),
                        )

                def x_side(ps, xT_sb, nkx, wi_sb):
                    zr, ih, rh = ps
                    for k in range(nkx):
                        xT = xT_sb[:, k * P : (k + 1) * P]
                        nc.tensor.matmul(
                            zr[:], xT, wi_sb[:, k * S : k * S + 2 * HS],
                            start=False, stop=(k == nkx - 1),
                        )
                        nc.tensor.matmul(
                            ih[:], xT, wi_sb[:, k * S + 2 * HS : (k + 1) * S],
                            start=False, stop=(k == nkx - 1),
                        )

                def gates(lname, ps, hsl, hn_out, out_dt):
                    zr, ih, rh = ps
                    z = apool.tile([B, HS], f32, tag="z", name=f"z_{lname}")
                    nc.scalar.activation(
                        z[:], zr[:, 0:HS], mybir.ActivationFunctionType.Sigmoid
                    )
                    r = apool.tile([B, HS], f32, tag="r", name=f"r_{lname}")
                    nc.scalar.activation(
                        r[:], zr[:, HS : 2 * HS],
                        mybir.ActivationFunctionType.Sigmoid,
                    )
                    t = apool.tile([B, HS], f32, tag="t", name=f"t_{lname}")
                    nc.vector.tensor_mul(t[:], r[:], rh[:])
                    u = apool.tile([B, HS], f32, tag="u", name=f"u_{lname}")
                    nc.vector.tensor_add(u[:], ih[:], t[:])
                    hh = apool.tile([B, HS], f32, tag="hh", name=f"hh_{lname}")
                    nc.scalar.activation(
                        hh[:], u[:], mybir.ActivationFunctionType.Tanh
                    )
                    d = apool.tile([B, HS], f32, tag="d", name=f"d_{lname}")
                    nc.vector.tensor_sub(d[:], hsl[:], hh[:])
                    zd = apool.tile([B, HS], f32, tag="zd", name=f"zd_{lname}")
                    nc.vector.tensor_mul(zd[:], z[:], d[:])
                    hn = apool.tile([B, HS], f32, tag="hn", name=f"hn_{lname}")
                    nc.vector.tensor_add(hn[:], hh[:], zd[:])
                    nc.sync.dma_start(hn_out[:], hn[:])
                    tr = psg.tile([P, HS], f32, tag="tr", bufs=1, name=f"tr_{lname}")
                    nc.tensor.transpose(tr[:], hn[:], ident32[:])
                    hnT = apool.tile([P, HS], out_dt, tag="hnT", name=f"hnT_{lname}")
                    nc.vector.tensor_copy(hnT[:], tr[:])
                    return hnT

                def ag_trigger(lname, hnT, dt):
                    # store + trigger both on the gpsimd queue: no cross-engine
                    # hop, and no queueing behind the bulk HWDGE stream
                    agi = dpool.tile(
                        [P, HS], dt, tag=f"agi_{lname}", name=f"agi_{lname}"
                    )
                    nc.gpsimd.dma_start(agi[:], hnT[:])
                    ago = dpool.tile(
                        [H, B], dt, tag=f"ago_{lname}", name=f"ago_{lname}",
                        addr_space="Shared",
                    )
                    nc.gpsimd.collective_compute(
                        "AllGather",
                        mybir.AluOpType.bypass,
                        replica_groups=RG,
                        ins=[agi[:].opt()],
                        outs=[ago[:].opt()],
                    )
                    return ago

                def ag_load(ago, dst_sb):
                    # one DMA: sbuf[p, k*B+b] = ago[k*128+p, b]
                    nc.sync.dma_start(
                        dst_sb.rearrange("p (k b) -> p k b", k=8),
                        ago[:].rearrange("(k p) b -> p k b", p=P),
                    )

                h0nT_sb = wpool.tile([P, H], gdt)
                h1nT_sb = wpool.tile([P, H], hdt)

                # ---- layer 0 (h-side first: eT arrives after the gather) ----
                ps0 = alloc_psums("l0")
                bias_mms(ps0, b0_sb)
                h_side(ps0, h0T_sb, wr0_sb)
                x_side(ps0, eT, 2, wi0_sb)
                hnT0 = gates("l0", ps0, h0sl, h0n_o, gdt)
                ago0 = ag_trigger("l0", hnT0, gdt)

                # ---- layer 1 h-side runs during AllGather 0 ----
                ps1 = alloc_psums("l1")
                bias_mms(ps1, b1_sb)
                h_side(ps1, h1T_sb, wr1_sb)
                ag_load(ago0, h0nT_sb[:])
                x_side(ps1, h0nT_sb, 8, wi1_sb)
                hnT1 = gates("l1", ps1, h1sl, h1n_o, hdt)
                ago1 = ag_trigger("l1", hnT1, hdt)
                ag_load(ago1, h1nT_sb[:])

            # ---- vocab head: logits_s = h1n @ ffW_s + ffb_s ----
            # Two k-outer halves: weights loaded 16x (not 64x), short copy tail.
            with tc.tile_pool(name="psh", bufs=4, space="PSUM") as psh:
                # keep-warm filler: spans the AllGather-1 wait so the PE's HAM
                # clock gate stays open for the head matmuls
                warm = psh.tile([B, 2 * HS], f32, tag="warm", bufs=1, name="warm")
                for w in range(56):
                    k = w % 8
                    nc.tensor.matmul(
                        warm[:],
                        h1T_sb[:, k * P : (k + 1) * P],
                        wr1_sb[:, k * S : k * S + 2 * HS],
                        start=True, stop=True,
                    )
                for half in range(2):
                    pls = []
                    for j in range(half * 4, half * 4 + 4):
                        pl = psh.tile([B, NTS], f32, tag="pl", name=f"pl{j}")
                        nc.tensor.matmul(
                            pl[:], ones_h[:1, :],
                            ffb_sb[:1, j * NTS : (j + 1) * NTS],
                            start=True, stop=False,
                        )
                        pls.append(pl)
                    for k in range(8):
                        h1nT_k = h1nT_sb[:, k * P : (k + 1) * P]
                        for jj, j in enumerate(range(half * 4, half * 4 + 4)):
                            nc.tensor.matmul(
                                pls[jj][:], h1nT_k,
                                fwp[:, k * VS + j * NTS : k * VS + (j + 1) * NTS],
                                start=False, stop=(k == 7),
                            )
                    for jj, j in enumerate(range(half * 4, half * 4 + 4)):
                        lo = opool.tile([B, NTS], f32, tag="lo", name=f"lo{j}")
                        nc.vector.tensor_copy(lo[:], pls[jj][:])
                        nc.sync.dma_start(
                            logits_o[:, j * NTS : (j + 1) * NTS], lo[:]
                        )

    nc.compile()
    return nc


def _make_runner(nc):
    """Build a jit'd 8-core executor with pre-placed device inputs.

    Mirrors concourse.bass2jax.run_bass_via_pjrt's multi-core path, but
    exposes input placement so host->device transfers are excluded from the
    measured NEFF execution (avoids cross-core launch skew from transfer
    completion times).
    """
    import jax
    from jax.sharding import Mesh, PartitionSpec, NamedSharding
    from jax.experimental.shard_map import shard_map
    from concourse import mybir
    import concourse.bass2jax as b2j

    b2j.install_neuronx_cc_hook()

    partition_name = (
        nc.partition_id_tensor.name if nc.partition_id_tensor else None
    )
    in_names, out_names, out_avals, zero_shapes = [], [], [], []
    for alloc in nc.m.functions[0].allocations:
        if not isinstance(alloc, mybir.MemoryLocationSet):
            continue
        name = alloc.memorylocations[0].name
        if alloc.kind == "ExternalInput":
            if name != partition_name:
                in_names.append(name)
        elif alloc.kind == "ExternalOutput":
            shape = tuple(alloc.tensor_shape)
            dtype = mybir.dt.np(alloc.dtype)
            out_names.append(name)
            out_avals.append(jax.core.ShapedArray(shape, dtype))
            zero_shapes.append((shape, dtype))
    n_params = len(in_names)
    n_outs = len(out_names)
    all_in_names = list(in_names) + list(out_names)
    if partition_name is not None:
        all_in_names.append(partition_name)

    donate = tuple(range(n_params, n_params + n_outs))

    def _body(*args):
        operands = list(args)
        if partition_name is not None:
            operands.append(b2j.partition_id_tensor())
        outs = b2j._bass_exec_p.bind(
            *operands,
            out_avals=tuple(out_avals),
            in_names=tuple(all_in_names),
            out_names=tuple(out_names),
            lowering_input_output_aliases=(),
            sim_require_finite=True,
            sim_require_nnan=True,
            nc=nc,
        )
        return tuple(outs)

    devices = jax.devices()[:NCORES]
    mesh = Mesh(np.asarray(devices), ("core",))
    in_specs = (PartitionSpec("core"),) * (n_params + n_outs)
    out_specs = (PartitionSpec("core"),) * n_outs
    sharded = jax.jit(
        shard_map(
            _body, mesh=mesh, in_specs=in_specs, out_specs=out_specs,
            check_rep=False,
        ),
        donate_argnums=donate,
        keep_unused=True,
    )
    sharding = NamedSharding(mesh, PartitionSpec("core"))

    def run(in_maps):
        import jax

        concat_in = [
            np.ascontiguousarray(
                np.concatenate(
                    [np.asarray(in_maps[c][n]) for c in range(NCORES)], axis=0
                )
            )
            for n in in_names
        ]
        dev_in = [jax.device_put(a, sharding) for a in concat_in]
        for a in dev_in:
            a.block_until_ready()

        def call():
            zeros = [
                jax.device_put(np.zeros((NCORES * s[0], *s[1:]), dt), sharding)
                for (s, dt) in zero_shapes
            ]
            for z in zeros:
                z.block_until_ready()
            outs = sharded(*dev_in, *zeros)
            for o in outs:
                o.block_until_ready()
            return outs

        call()  # warmup: compile + first NEFF execution
        outs = call()
        res = [
            {
                name: np.asarray(outs[i]).reshape(NCORES, *out_avals[i].shape)[c]
                for i, name in enumerate(out_names)
            }
            for c in range(NCORES)
        ]
        return res, call

    return run


def _get():
    global _built
    if _built is None:
        gdt_np, hdt_np = _dtypes()
        nc = _build_kernel(gdt_np, hdt_np)
        runner = _make_runner(nc)
        _built = (nc, runner, (gdt_np, hdt_np))
    return _built


def kernel(**inputs):
    nc, runner, dts = _get()
    in_maps = _prepare_in_maps(inputs, dts)
    res, _ = runner(in_maps)
    return _assemble(res)


def _prepare_in_maps(inputs, dts):
    gdt_np, hdt_np = dts
    f32 = np.float32
    x = np.asarray(inputs["x"]).astype(np.int32).reshape(B, 1)
    h0 = np.asarray(inputs["h0"], dtype=f32)
    h1 = np.asarray(inputs["h1"], dtype=f32)
    emb = np.asarray(inputs["emb"], dtype=f32)
    Wi0 = np.asarray(inputs["Wi0"], dtype=f32)
    Wr0 = np.asarray(inputs["Wr0"], dtype=f32)
    bi0 = np.asarray(inputs["bi0"], dtype=f32)
    br0 = np.asarray(inputs["br0"], dtype=f32)
    Wi1 = np.asarray(inputs["Wi1"], dtype=f32)
    Wr1 = np.asarray(inputs["Wr1"], dtype=f32)
    bi1 = np.asarray(inputs["bi1"], dtype=f32)
    br1 = np.asarray(inputs["br1"], dtype=f32)
    ffW = np.asarray(inputs["ffW"], dtype=f32)
    ffb = np.asarray(inputs["ffb"], dtype=f32)

    g = lambda a: np.ascontiguousarray(a).astype(gdt_np)
    hc = lambda a: np.ascontiguousarray(a).astype(hdt_np)
    emb_g = g(emb)
    hT2 = g(np.concatenate([_pack_hT(h0), _pack_hT(h1)], axis=1))

    def layer_bias(bi, br, i):
        bsum = bi + br
        bz = bsum[0 * H + i * HS : 0 * H + (i + 1) * HS]
        br_ = bsum[1 * H + i * HS : 1 * H + (i + 1) * HS]
        bih = bi[2 * H + i * HS : 2 * H + (i + 1) * HS]
        brh = br[2 * H + i * HS : 2 * H + (i + 1) * HS]
        return np.concatenate([bz, br_, bih, brh])

    in_maps = []
    for i in range(NCORES):
        gw0 = np.concatenate(
            [_pack_w(_gate_cols(Wi0, i)), _pack_w(_gate_cols(Wr0, i))], axis=1
        )
        gw1 = np.concatenate(
            [_pack_w(_gate_cols(Wi1, i)), _pack_w(_gate_cols(Wr1, i))], axis=1
        )
        b01 = np.concatenate(
            [layer_bias(bi0, br0, i), layer_bias(bi1, br1, i)]
        )[None, :]
        in_maps.append(
            {
                "x_ids": x,
                "emb": emb_g,
                "gw0": g(gw0),
                "gw1": g(gw1),
                "hT2": hT2,
                "hsl2": np.ascontiguousarray(
                    np.concatenate(
                        [
                            h0[:, i * HS : (i + 1) * HS],
                            h1[:, i * HS : (i + 1) * HS],
                        ],
                        axis=1,
                    )
                ),
                "b01": g(b01),
                "ffb": hc(ffb[i * VS : (i + 1) * VS][None, :]),
                "fwp": hc(_pack_w(ffW[:, i * VS : (i + 1) * VS])),
            }
        )
    return in_maps


def _assemble(res):
    logits = np.concatenate([res[c]["logits_s"] for c in range(NCORES)], axis=1)
    h0n = np.concatenate([res[c]["h0n_s"] for c in range(NCORES)], axis=1)
    h1n = np.concatenate([res[c]["h1n_s"] for c in range(NCORES)], axis=1)
    return logits, h0n, h1n
